# revision 16
# baseline (speedup 1.0000x reference)
"""Trainium2 Bass kernel for a SqueezeNet Fire module.

    x [32, 512, 56, 56] fp32
    s  = relu(squeeze_w @ x + squeeze_b)          # 1x1, 512 -> 64
    e1 = relu(expand1x1_w @ s + expand1x1_b)      # 1x1, 64 -> 256
    e3 = relu(conv3x3(s, expand3x3_w) + b)        # 3x3 pad 1, 64 -> 256
    out = concat([e1, e3], channel)               # [32, 512, 56, 56] fp32

Sharding: data-parallel over batch, 4 images per NeuronCore x 8 cores.

Per-core plan (per image, spatial flattened to 56x56=3136, chunked 7x448):
  - squeeze: 4 accumulating K=128 matmuls. The stationary weights are
    duplicated along M (64 real channels -> 128) so PSUM rows 0-63 and 64-127
    both hold S; one scalar-engine relu+bias eviction writes both halves of a
    zero-padded S buffer SS [128, 58, 58] (partitions 0-63 = copy A,
    64-127 = copy B).
  - expand1x1 / expand3x3: K=64 matmuls issued as pairs on row groups 0-63 and
    64-127 (auto tile_position from base_partition) so each pair runs
    concurrently in the PE array. expand3x3 = 9 shifted-tap matmuls
    accumulating in PSUM, taps read shifted windows of SS.
  - evictions fused bias+relu: scalar engine for squeeze + e3, vector engine
    (tensor_scalar add/max) for e1.

The kernel is HBM-traffic-bound (x in + out out), so I/O is staged in bf16
(x cast on host, output upcast on host) when the *_BF16 flags are set;
matmul operands are bf16 (squeeze) / float32r (expands, TF32-like) with fp32
PSUM accumulation.
"""

import sys

if "/opt/trn_rl_repo" not in sys.path:
    sys.path.insert(0, "/opt/trn_rl_repo")

import ml_dtypes
import numpy as np

import concourse.bass as bass
import concourse.tile as tile
from concourse import bacc, mybir

F32 = mybir.dt.float32
F32R = mybir.dt.float32r
BF16 = mybir.dt.bfloat16
RELU = mybir.ActivationFunctionType.Relu

N_CORES = 8
N_TOTAL, C_IN, H, W = 32, 512, 56, 56
N_IMG = N_TOTAL // N_CORES          # images per core
C_SQ, C_E = 64, 256                 # squeeze / expand channels
HW = H * W                          # 3136
ROWS_PER_CHUNK = 8
N_CHUNK = H // ROWS_PER_CHUNK       # 7 chunks of 8 rows
CHUNK = ROWS_PER_CHUNK * W          # 448 spatial positions per chunk
HP, WP = H + 2, W + 2               # padded S frame 58x58
K_TILES = C_IN // 128               # 4

IN_BF16 = True                      # ship x to the device as bf16
EXP_BF16 = True                     # expand path (S buffer + weights) in bf16
OUT_BF16 = False                    # write out as bf16, upcast on host


def _build(in_bf16, exp_bf16, out_bf16):
    xdt = BF16 if in_bf16 else F32R
    edt = BF16 if exp_bf16 else F32R
    odt = BF16 if out_bf16 else F32
    nc = bacc.Bacc("TRN2", target_bir_lowering=False, debug=False,
                   num_devices=N_CORES)
    x_d = nc.dram_tensor("x", [N_IMG, 128, K_TILES, HW], xdt,
                         kind="ExternalInput").ap()
    wsq_d = nc.dram_tensor("wsq", [128, K_TILES, C_SQ], xdt,
                           kind="ExternalInput").ap()
    w1_d = nc.dram_tensor("w1", [128, 128], edt, kind="ExternalInput").ap()
    w3_d = nc.dram_tensor("w3", [128, 9, 128], edt, kind="ExternalInput").ap()
    bsq_d = nc.dram_tensor("bsq", [128, 1], F32, kind="ExternalInput").ap()
    b1_d = nc.dram_tensor("b1", [128, 2], F32, kind="ExternalInput").ap()
    b3_d = nc.dram_tensor("b3", [128, 2], F32, kind="ExternalInput").ap()
    out_d = nc.dram_tensor("out", [N_IMG, 2 * C_E, HW], odt,
                           kind="ExternalOutput").ap()

    with tile.TileContext(nc) as tc:
        with (
            tc.tile_pool(name="wpool", bufs=1) as wpool,
            tc.tile_pool(name="xpool", bufs=5) as xpool,
            tc.tile_pool(name="sspool", bufs=2) as sspool,
            tc.tile_pool(name="opool", bufs=3) as opool,
            tc.tile_pool(name="psum", bufs=1, space="PSUM") as psum,
        ):
            wsq_t = wpool.tile([128, K_TILES, C_SQ], xdt)
            w1_t = wpool.tile([128, 128], edt)
            w3_t = wpool.tile([128, 9, 128], edt)
            bsq_t = wpool.tile([128, 1], F32)
            b1_t = wpool.tile([128, 2], F32)
            b3_t = wpool.tile([128, 2], F32)
            nc.sync.dma_start(wsq_t[:], wsq_d[:])
            nc.sync.dma_start(w1_t[:], w1_d[:])
            nc.sync.dma_start(w3_t[:], w3_d[:])
            nc.sync.dma_start(bsq_t[:], bsq_d[:])
            nc.sync.dma_start(b1_t[:], b1_d[:])
            nc.sync.dma_start(b3_t[:], b3_d[:])

            x_tiles = {}    # (image, chunk group) -> [128, K_TILES, 2*CHUNK]
            ss_tiles = {}   # image -> SS tile
            out_stage = [None] * 4

            def load_group(n, g, eng=None):
                # one DMA covers all 4 K-tiles of a chunk group (2 chunks,
                # or 1 for the image's odd last chunk); issued on the gpsimd
                # (SWDGE) queue so a slot-wait here never head-of-line-blocks
                # the sync (HWDGE) output DMAs. The first groups go on sync
                # (hardware DGE) instead: it starts transfers ~3us sooner
                # out of the preamble, and there are no output DMAs yet to
                # block.
                w = min(2 * CHUNK, HW - 2 * g * CHUNK)
                t = xpool.tile([128, K_TILES, w], xdt, tag="xc",
                               name=f"xc_{n}_{g}")
                (eng or nc.gpsimd).dma_start(
                    t[:], x_d[n, :, :, 2 * g * CHUNK : 2 * g * CHUNK + w]
                )
                x_tiles[(n, g)] = t

            def setup_image(n):
                ss = sspool.tile([128, HP, WP], edt, tag="ss")
                # zero the one-pixel border of the padded S frame (memset
                # rejects the f32r dtype tag, so write through a plain view)
                mdt = BF16 if exp_bf16 else F32
                nc.vector.memset(ss[:, 0, :].bitcast(mdt), 0.0)
                nc.vector.memset(ss[:, HP - 1, :].bitcast(mdt), 0.0)
                nc.vector.memset(ss[:, 1 : HP - 1, 0].bitcast(mdt), 0.0)
                nc.vector.memset(ss[:, 1 : HP - 1, WP - 1].bitcast(mdt), 0.0)
                ss_tiles[n] = ss

            def squeeze_pair(n, g):
                # Column-paired squeeze: chunk 2g lands on PSUM partitions
                # 0-63 (col group 0) while chunk 2g+1 lands on partitions
                # 64-127 (col group 64) of a concurrent matmul pair — 4 PE
                # slots produce TWO chunks of S. The two halves accumulate in
                # different PSUM banks (same bank would share one zero
                # region). S is then mirrored onto the other partition half
                # with an SBUF->SBUF DMA so the expand row-pairing still sees
                # identical copies on both halves.
                if n not in ss_tiles:
                    setup_image(n)
                ss = ss_tiles[n]
                c0 = 2 * g
                cw = 1 if c0 == N_CHUNK - 1 else 2
                # [128, 2, 512]: each half padded to exactly one PSUM bank
                # so the two col-group accumulations sit in different banks
                ps = psum.tile([128, 2, 512], F32, tag="sq",
                               bufs=1, name=f"sq_{n}_{g}")
                xt = x_tiles[(n, g)]
                for k in range(K_TILES):
                    nc.tensor.matmul(
                        ps[0:64, 0, 0:CHUNK],
                        wsq_t[:, k, :],
                        xt[:, k, 0:CHUNK],
                        start=(k == 0),
                        stop=(k == K_TILES - 1),
                    )
                    if cw == 2:
                        nc.tensor.matmul(
                            ps[64:128, 1, 0:CHUNK],
                            wsq_t[:, k, :],
                            xt[:, k, CHUNK : 2 * CHUNK],
                            start=(k == 0),
                            stop=(k == K_TILES - 1),
                        )
                r0 = 1 + c0 * ROWS_PER_CHUNK
                nc.scalar.activation(
                    ss[0:64, r0 : r0 + ROWS_PER_CHUNK, 1 : 1 + W],
                    ps[0:64, 0, 0:CHUNK].rearrange(
                        "p (a b) -> p a b", a=ROWS_PER_CHUNK),
                    RELU,
                    bias=bsq_t[0:64, :],
                )
                nc.gpsimd.dma_start(
                    ss[64:128, r0 : r0 + ROWS_PER_CHUNK, 1 : 1 + W],
                    ss[0:64, r0 : r0 + ROWS_PER_CHUNK, 1 : 1 + W],
                )
                if cw == 2:
                    r1 = r0 + ROWS_PER_CHUNK
                    nc.scalar.activation(
                        ss[64:128, r1 : r1 + ROWS_PER_CHUNK, 1 : 1 + W],
                        ps[64:128, 1, 0:CHUNK].rearrange(
                            "p (a b) -> p a b", a=ROWS_PER_CHUNK),
                        RELU,
                        bias=bsq_t[64:128, :],
                    )
                    nc.gpsimd.dma_start(
                        ss[0:64, r1 : r1 + ROWS_PER_CHUNK, 1 : 1 + W],
                        ss[64:128, r1 : r1 + ROWS_PER_CHUNK, 1 : 1 + W],
                    )

            e_state = {}

            def expand_chunk_mm(n, j, taps):
                # taps: range of expand3x3 tap indices to emit now. Callers
                # split chunk 2g+1's taps so the dy>=1 taps (which read S
                # rows of pair g+1) are emitted late, hiding the eviction +
                # mirror-copy chain of that pair behind earlier PE work.
                ss = ss_tiles[n]
                y0 = j * ROWS_PER_CHUNK
                if taps[0] == 0:
                    p1 = [psum.tile([128, CHUNK], F32, tag=f"e1h{h}", bufs=1,
                                    name=f"p1h{h}_{n}_{j}")
                          for h in range(2)]
                    p3 = [psum.tile([128, CHUNK], F32, tag=f"e3h{h}", bufs=2,
                                    name=f"p3h{h}_{n}_{j}")
                          for h in range(2)]
                    e_state[(n, j)] = (p1, p3)
                    # expand1x1: one K=64 matmul per half, concurrent pair
                    for h in range(2):
                        nc.tensor.matmul(
                            p1[h][:],
                            w1_t[64 * h : 64 * h + 64, :],
                            ss[64 * h : 64 * h + 64,
                               1 + y0 : 1 + y0 + ROWS_PER_CHUNK, 1 : 1 + W],
                            start=True,
                            stop=True,
                        )
                p1, p3 = e_state[(n, j)]
                # expand3x3: 9 shifted taps accumulate; h0/h1 issued as pairs
                for t in taps:
                    dy, dx = t // 3, t % 3
                    for h in range(2):
                        nc.tensor.matmul(
                            p3[h][:],
                            w3_t[64 * h : 64 * h + 64, t, :],
                            ss[64 * h : 64 * h + 64,
                               y0 + dy : y0 + dy + ROWS_PER_CHUNK,
                               dx : dx + W],
                            start=(t == 0),
                            stop=(t == 8),
                        )
                if taps[-1] != 8:
                    return

            def expand_chunk_evict(n, j):
                p1, p3 = e_state.pop((n, j))
                # evictions: e1 on vector engine, e3 on scalar engine.
                # Outputs stage in 2-chunk tiles; one DMA per role per pair
                # of chunks (issued after the odd chunk's eviction).
                ss = ss_tiles
                g, half = j // 2, j % 2
                gw = 1 if j == N_CHUNK - 1 else 2   # odd last chunk: solo group
                if half == 0:
                    for role in range(4):
                        out_stage[role] = opool.tile(
                            [128, gw, CHUNK], odt, tag=f"o{role}",
                            name=f"o{role}_{n}_{g}")
                for h in range(2):
                    nc.vector.tensor_scalar(
                        out_stage[h][:, half, :], p1[h][:],
                        b1_t[:, h : h + 1], 0.0,
                        op0=mybir.AluOpType.add, op1=mybir.AluOpType.max,
                    )
                for h in range(2):
                    nc.scalar.activation(out_stage[2 + h][:, half, :],
                                         p3[h][:], RELU,
                                         bias=b3_t[:, h : h + 1])
                if half + 1 == gw:
                    for role in range(4):
                        ch0 = 128 * role
                        nc.sync.dma_start(
                            out_d[n, ch0 : ch0 + 128,
                                  2 * g * CHUNK : (2 * g + gw) * CHUNK],
                            out_stage[role][:],
                        )

            # Pipeline: squeeze runs two chunks ahead of expand — expand(i)'s
            # dy=2 taps read S rows that squeeze(i+1)'s eviction writes, so
            # squeeze(i+1) must have been evicted; running squeeze(i+2) keeps
            # the PE busy during that eviction. x is prefetched PREFETCH
            # chunks ahead at chunk granularity so the pipeline never stalls
            # on an image-sized transfer.
            PREFETCH = 3            # x prefetch depth, in 2-chunk groups
            groups = []
            for n in range(N_IMG):
                for g in range((N_CHUNK + 1) // 2):
                    groups.append((n, g))
            for gi in range(min(PREFETCH, len(groups))):
                load_group(*groups[gi])
            next_load = PREFETCH
            squeeze_pair(*groups[0])
            ALL = list(range(9))
            for i, (n, g) in enumerate(groups):
                if i + 1 < len(groups):
                    if next_load < len(groups):
                        load_group(*groups[next_load])
                        next_load += 1
                    squeeze_pair(*groups[i + 1])
                c0 = 2 * g
                cw = 1 if c0 == N_CHUNK - 1 else 2
                if cw == 1:
                    expand_chunk_mm(n, c0, ALL)
                    expand_chunk_evict(n, c0)
                else:
                    expand_chunk_mm(n, c0, ALL)
                    expand_chunk_evict(n, c0)
                    expand_chunk_mm(n, c0 + 1, list(range(3)))
                    expand_chunk_mm(n, c0 + 1, list(range(3, 9)))
                    expand_chunk_evict(n, c0 + 1)

    nc.compile()
    return nc


_NC_CACHE = {}


def _get_nc(in_bf16=IN_BF16, exp_bf16=EXP_BF16, out_bf16=OUT_BF16):
    key = (in_bf16, exp_bf16, out_bf16)
    if key not in _NC_CACHE:
        _NC_CACHE[key] = _build(in_bf16, exp_bf16, out_bf16)
    return _NC_CACHE[key]


def _pack_inputs(x, squeeze_w, squeeze_b, expand1x1_w, expand1x1_b,
                 expand3x3_w, expand3x3_b, in_bf16=IN_BF16, exp_bf16=EXP_BF16):
    """Host-side packing of weights into the SBUF-ready layouts."""
    f = np.float32
    xdt = ml_dtypes.bfloat16 if in_bf16 else f
    edt = ml_dtypes.bfloat16 if exp_bf16 else f
    # wsq[p, k, m] = squeeze_w[m, 128k + p]
    wsq = np.ascontiguousarray(
        squeeze_w.T.reshape(K_TILES, 128, C_SQ)    # [k, p, m]
        .transpose(1, 0, 2)
    ).astype(xdt)
    # w1[64h + s, m] = expand1x1_w[128h + m, s]
    w1 = np.concatenate(
        [expand1x1_w[:128].T, expand1x1_w[128:].T], axis=0
    ).astype(edt)                                   # [128, 128]
    # w3[64h + s, 3dy + dx, m] = expand3x3_w[128h + m, s, dy, dx]
    w3e = expand3x3_w.reshape(2, 128, C_SQ, 9)      # [h, m, s, t]
    w3 = np.ascontiguousarray(w3e.transpose(0, 2, 3, 1)).reshape(128, 9, 128)
    w3 = w3.astype(edt)
    bsq = np.tile(squeeze_b, 2).reshape(128, 1).astype(f)
    b1 = np.ascontiguousarray(expand1x1_b.reshape(2, 128).T).astype(f)
    b3 = np.ascontiguousarray(expand3x3_b.reshape(2, 128).T).astype(f)
    # [cores, n, 128k+p, s] -> [cores, n, p, k, s] so a chunk-group load is
    # one DMA with partition-major layout
    xs = np.ascontiguousarray(
        x.reshape(N_CORES, N_IMG, K_TILES, 128, HW).transpose(0, 1, 3, 2, 4)
    ).astype(xdt)
    return xs, {"wsq": wsq, "w1": w1, "w3": w3, "bsq": bsq, "b1": b1, "b3": b3}


def _run(inputs, trace=False, in_bf16=IN_BF16, exp_bf16=EXP_BF16,
         out_bf16=OUT_BF16):
    from concourse import bass_utils

    nc = _get_nc(in_bf16, exp_bf16, out_bf16)
    xs, weights = _pack_inputs(**inputs, in_bf16=in_bf16, exp_bf16=exp_bf16)
    in_maps = [{"x": xs[c], **weights} for c in range(N_CORES)]
    res = bass_utils.run_bass_kernel_spmd(
        nc, in_maps, core_ids=list(range(N_CORES)), trace=trace
    )
    out = np.concatenate([res.results[c]["out"] for c in range(N_CORES)], axis=0)
    return out.reshape(N_TOTAL, 2 * C_E, H, W).astype(np.float32), res


def kernel(**inputs) -> np.ndarray:
    inputs = {k: np.asarray(v, dtype=np.float32) for k, v in inputs.items()}
    out, _ = _run(inputs, trace=False)
    return out


# revision 17
# speedup vs baseline: 1.3884x; 1.3884x over previous
"""Trainium2 Bass kernel for a SqueezeNet Fire module.

    x [32, 512, 56, 56] fp32
    s  = relu(squeeze_w @ x + squeeze_b)          # 1x1, 512 -> 64
    e1 = relu(expand1x1_w @ s + expand1x1_b)      # 1x1, 64 -> 256
    e3 = relu(conv3x3(s, expand3x3_w) + b)        # 3x3 pad 1, 64 -> 256
    out = concat([e1, e3], channel)               # [32, 512, 56, 56] fp32

Sharding: data-parallel over batch, 4 images per NeuronCore x 8 cores.

Per-core plan (per image, spatial flattened to 56x56=3136, chunked 7x448):
  - squeeze: 4 accumulating K=128 matmuls. The stationary weights are
    duplicated along M (64 real channels -> 128) so PSUM rows 0-63 and 64-127
    both hold S; one scalar-engine relu+bias eviction writes both halves of a
    zero-padded S buffer SS [128, 58, 58] (partitions 0-63 = copy A,
    64-127 = copy B).
  - expand1x1 / expand3x3: K=64 matmuls issued as pairs on row groups 0-63 and
    64-127 (auto tile_position from base_partition) so each pair runs
    concurrently in the PE array. expand3x3 = 9 shifted-tap matmuls
    accumulating in PSUM, taps read shifted windows of SS.
  - evictions fused bias+relu: scalar engine for squeeze + e3, vector engine
    (tensor_scalar add/max) for e1.

The kernel is HBM-traffic-bound (x in + out out), so I/O is staged in bf16
(x cast on host, output upcast on host) when the *_BF16 flags are set;
matmul operands are bf16 (squeeze) / float32r (expands, TF32-like) with fp32
PSUM accumulation.
"""

import sys

if "/opt/trn_rl_repo" not in sys.path:
    sys.path.insert(0, "/opt/trn_rl_repo")

import ml_dtypes
import numpy as np

import concourse.bass as bass
import concourse.tile as tile
from concourse import bacc, mybir

F32 = mybir.dt.float32
F32R = mybir.dt.float32r
BF16 = mybir.dt.bfloat16
RELU = mybir.ActivationFunctionType.Relu

N_CORES = 8
N_TOTAL, C_IN, H, W = 32, 512, 56, 56
N_IMG = N_TOTAL // N_CORES          # images per core
C_SQ, C_E = 64, 256                 # squeeze / expand channels
HW = H * W                          # 3136
ROWS_PER_CHUNK = 8
N_CHUNK = H // ROWS_PER_CHUNK       # 7 chunks of 8 rows
CHUNK = ROWS_PER_CHUNK * W          # 448 spatial positions per chunk
HP, WP = H + 2, W + 2               # padded S frame 58x58
K_TILES = C_IN // 128               # 4

IN_BF16 = True                      # ship x to the device as bf16
EXP_BF16 = True                     # expand path (S buffer + weights) in bf16
OUT_BF16 = False                    # write out as bf16, upcast on host


def _build(in_bf16, exp_bf16, out_bf16):
    xdt = BF16 if in_bf16 else F32R
    edt = BF16 if exp_bf16 else F32R
    odt = BF16 if out_bf16 else F32
    nc = bacc.Bacc("TRN2", target_bir_lowering=False, debug=False,
                   num_devices=N_CORES)
    x_d = nc.dram_tensor("x", [N_IMG, 128, K_TILES, HW], xdt,
                         kind="ExternalInput").ap()
    wsq_d = nc.dram_tensor("wsq", [128, K_TILES, C_SQ], xdt,
                           kind="ExternalInput").ap()
    w1_d = nc.dram_tensor("w1", [128, 128], edt, kind="ExternalInput").ap()
    w3_d = nc.dram_tensor("w3", [128, 9, 128], edt, kind="ExternalInput").ap()
    bsq_d = nc.dram_tensor("bsq", [128, 1], F32, kind="ExternalInput").ap()
    b1_d = nc.dram_tensor("b1", [128, 2], F32, kind="ExternalInput").ap()
    b3_d = nc.dram_tensor("b3", [128, 2], F32, kind="ExternalInput").ap()
    out_d = nc.dram_tensor("out", [N_IMG, 2 * C_E, HW], odt,
                           kind="ExternalOutput").ap()

    with tile.TileContext(nc) as tc:
        with (
            tc.tile_pool(name="wpool", bufs=1) as wpool,
            tc.tile_pool(name="xpool", bufs=5) as xpool,
            tc.tile_pool(name="sspool", bufs=2) as sspool,
            tc.tile_pool(name="opool", bufs=3) as opool,
            tc.tile_pool(name="psum", bufs=1, space="PSUM") as psum,
        ):
            wsq_t = wpool.tile([128, K_TILES, C_SQ], xdt)
            w1_t = wpool.tile([128, 128], edt)
            w3_t = wpool.tile([128, 9, 128], edt)
            bsq_t = wpool.tile([128, 1], F32)
            b1_t = wpool.tile([128, 2], F32)
            b3_t = wpool.tile([128, 2], F32)
            nc.sync.dma_start(wsq_t[:], wsq_d[:])
            nc.sync.dma_start(w1_t[:], w1_d[:])
            nc.sync.dma_start(w3_t[:], w3_d[:])
            nc.sync.dma_start(bsq_t[:], bsq_d[:])
            nc.sync.dma_start(b1_t[:], b1_d[:])
            nc.sync.dma_start(b3_t[:], b3_d[:])

            x_tiles = {}    # (image, chunk group) -> [128, K_TILES, 2*CHUNK]
            ss_tiles = {}   # image -> SS tile
            out_stage = [None] * 4

            def load_group(n, g, eng=None):
                # one DMA covers all 4 K-tiles of a chunk group (2 chunks,
                # or 1 for the image's odd last chunk); issued on the gpsimd
                # (SWDGE) queue so a slot-wait here never head-of-line-blocks
                # the sync (HWDGE) output DMAs. The first groups go on sync
                # (hardware DGE) instead: it starts transfers ~3us sooner
                # out of the preamble, and there are no output DMAs yet to
                # block.
                w = min(2 * CHUNK, HW - 2 * g * CHUNK)
                t = xpool.tile([128, K_TILES, w], xdt, tag="xc",
                               name=f"xc_{n}_{g}")
                (eng or nc.gpsimd).dma_start(
                    t[:], x_d[n, :, :, 2 * g * CHUNK : 2 * g * CHUNK + w]
                )
                x_tiles[(n, g)] = t

            def setup_image(n):
                ss = sspool.tile([128, HP, WP], edt, tag="ss")
                # zero the one-pixel border of the padded S frame (memset
                # rejects the f32r dtype tag, so write through a plain view)
                mdt = BF16 if exp_bf16 else F32
                nc.vector.memset(ss[:, 0, :].bitcast(mdt), 0.0)
                nc.vector.memset(ss[:, HP - 1, :].bitcast(mdt), 0.0)
                nc.vector.memset(ss[:, 1 : HP - 1, 0].bitcast(mdt), 0.0)
                nc.vector.memset(ss[:, 1 : HP - 1, WP - 1].bitcast(mdt), 0.0)
                ss_tiles[n] = ss

            def squeeze_pair(n, g):
                # Column-paired squeeze: chunk 2g lands on PSUM partitions
                # 0-63 (col group 0) while chunk 2g+1 lands on partitions
                # 64-127 (col group 64) of a concurrent matmul pair — 4 PE
                # slots produce TWO chunks of S. The two halves accumulate in
                # different PSUM banks (same bank would share one zero
                # region). S is then mirrored onto the other partition half
                # with an SBUF->SBUF DMA so the expand row-pairing still sees
                # identical copies on both halves.
                if n not in ss_tiles:
                    setup_image(n)
                ss = ss_tiles[n]
                c0 = 2 * g
                cw = 1 if c0 == N_CHUNK - 1 else 2
                # [128, 2, 512]: each half padded to exactly one PSUM bank
                # so the two col-group accumulations sit in different banks
                ps = psum.tile([128, 2, 512], F32, tag="sq",
                               bufs=2, name=f"sq_{n}_{g}")
                xt = x_tiles[(n, g)]
                for k in range(K_TILES):
                    nc.tensor.matmul(
                        ps[0:64, 0, 0:CHUNK],
                        wsq_t[:, k, :],
                        xt[:, k, 0:CHUNK],
                        start=(k == 0),
                        stop=(k == K_TILES - 1),
                    )
                    if cw == 2:
                        nc.tensor.matmul(
                            ps[64:128, 1, 0:CHUNK],
                            wsq_t[:, k, :],
                            xt[:, k, CHUNK : 2 * CHUNK],
                            start=(k == 0),
                            stop=(k == K_TILES - 1),
                        )
                r0 = 1 + c0 * ROWS_PER_CHUNK
                nc.scalar.activation(
                    ss[0:64, r0 : r0 + ROWS_PER_CHUNK, 1 : 1 + W],
                    ps[0:64, 0, 0:CHUNK].rearrange(
                        "p (a b) -> p a b", a=ROWS_PER_CHUNK),
                    RELU,
                    bias=bsq_t[0:64, :],
                )
                nc.vector.stream_shuffle(
                    ss[64:128, r0 : r0 + ROWS_PER_CHUNK, 1 : 1 + W],
                    ss[0:64, r0 : r0 + ROWS_PER_CHUNK, 1 : 1 + W],
                    mask=list(range(32)),
                )
                if cw == 2:
                    r1 = r0 + ROWS_PER_CHUNK
                    nc.scalar.activation(
                        ss[64:128, r1 : r1 + ROWS_PER_CHUNK, 1 : 1 + W],
                        ps[64:128, 1, 0:CHUNK].rearrange(
                            "p (a b) -> p a b", a=ROWS_PER_CHUNK),
                        RELU,
                        bias=bsq_t[64:128, :],
                    )
                    nc.vector.stream_shuffle(
                        ss[0:64, r1 : r1 + ROWS_PER_CHUNK, 1 : 1 + W],
                        ss[64:128, r1 : r1 + ROWS_PER_CHUNK, 1 : 1 + W],
                        mask=list(range(32)),
                    )

            e_state = {}

            def expand_chunk_mm(n, j, taps):
                # taps: range of expand3x3 tap indices to emit now. Callers
                # split chunk 2g+1's taps so the dy>=1 taps (which read S
                # rows of pair g+1) are emitted late, hiding the eviction +
                # mirror-copy chain of that pair behind earlier PE work.
                ss = ss_tiles[n]
                y0 = j * ROWS_PER_CHUNK
                if taps[0] == 0:
                    p1 = [psum.tile([128, CHUNK], F32, tag=f"e1h{h}", bufs=1,
                                    name=f"p1h{h}_{n}_{j}")
                          for h in range(2)]
                    p3 = [psum.tile([128, CHUNK], F32, tag=f"e3h{h}", bufs=1,
                                    name=f"p3h{h}_{n}_{j}")
                          for h in range(2)]
                    e_state[(n, j)] = (p1, p3)
                    # expand1x1: one K=64 matmul per half, concurrent pair
                    for h in range(2):
                        nc.tensor.matmul(
                            p1[h][:],
                            w1_t[64 * h : 64 * h + 64, :],
                            ss[64 * h : 64 * h + 64,
                               1 + y0 : 1 + y0 + ROWS_PER_CHUNK, 1 : 1 + W],
                            start=True,
                            stop=True,
                        )
                p1, p3 = e_state[(n, j)]
                # expand3x3: 9 shifted taps accumulate; h0/h1 issued as pairs
                for t in taps:
                    dy, dx = t // 3, t % 3
                    for h in range(2):
                        nc.tensor.matmul(
                            p3[h][:],
                            w3_t[64 * h : 64 * h + 64, t, :],
                            ss[64 * h : 64 * h + 64,
                               y0 + dy : y0 + dy + ROWS_PER_CHUNK,
                               dx : dx + W],
                            start=(t == 0),
                            stop=(t == 8),
                        )
                if taps[-1] != 8:
                    return

            def expand_chunk_evict(n, j):
                p1, p3 = e_state.pop((n, j))
                # evictions: e1 on vector engine, e3 on scalar engine.
                # Outputs stage in 2-chunk tiles; one DMA per role per pair
                # of chunks (issued after the odd chunk's eviction).
                ss = ss_tiles
                g, half = j // 2, j % 2
                gw = 1 if j == N_CHUNK - 1 else 2   # odd last chunk: solo group
                if half == 0:
                    for role in range(4):
                        out_stage[role] = opool.tile(
                            [128, gw, CHUNK], odt, tag=f"o{role}",
                            name=f"o{role}_{n}_{g}")
                for h in range(2):
                    nc.vector.tensor_scalar(
                        out_stage[h][:, half, :], p1[h][:],
                        b1_t[:, h : h + 1], 0.0,
                        op0=mybir.AluOpType.add, op1=mybir.AluOpType.max,
                    )
                for h in range(2):
                    nc.scalar.activation(out_stage[2 + h][:, half, :],
                                         p3[h][:], RELU,
                                         bias=b3_t[:, h : h + 1])
                if half + 1 == gw:
                    for role in range(4):
                        ch0 = 128 * role
                        nc.sync.dma_start(
                            out_d[n, ch0 : ch0 + 128,
                                  2 * g * CHUNK : (2 * g + gw) * CHUNK],
                            out_stage[role][:],
                        )

            # Pipeline: squeeze runs two chunks ahead of expand — expand(i)'s
            # dy=2 taps read S rows that squeeze(i+1)'s eviction writes, so
            # squeeze(i+1) must have been evicted; running squeeze(i+2) keeps
            # the PE busy during that eviction. x is prefetched PREFETCH
            # chunks ahead at chunk granularity so the pipeline never stalls
            # on an image-sized transfer.
            PREFETCH = 3            # x prefetch depth, in 2-chunk groups
            groups = []
            for n in range(N_IMG):
                for g in range((N_CHUNK + 1) // 2):
                    groups.append((n, g))
            for gi in range(min(PREFETCH, len(groups))):
                load_group(*groups[gi])
            next_load = PREFETCH
            squeeze_pair(*groups[0])
            squeeze_pair(*groups[1])
            ALL = list(range(9))
            for i, (n, g) in enumerate(groups):
                if i + 2 < len(groups):
                    if next_load < len(groups):
                        load_group(*groups[next_load])
                        next_load += 1
                    squeeze_pair(*groups[i + 2])
                c0 = 2 * g
                cw = 1 if c0 == N_CHUNK - 1 else 2
                if cw == 1:
                    expand_chunk_mm(n, c0, ALL)
                    expand_chunk_evict(n, c0)
                else:
                    expand_chunk_mm(n, c0, ALL)
                    expand_chunk_evict(n, c0)
                    expand_chunk_mm(n, c0 + 1, list(range(3)))
                    expand_chunk_mm(n, c0 + 1, list(range(3, 9)))
                    expand_chunk_evict(n, c0 + 1)

    nc.compile()
    return nc


_NC_CACHE = {}


def _get_nc(in_bf16=IN_BF16, exp_bf16=EXP_BF16, out_bf16=OUT_BF16):
    key = (in_bf16, exp_bf16, out_bf16)
    if key not in _NC_CACHE:
        _NC_CACHE[key] = _build(in_bf16, exp_bf16, out_bf16)
    return _NC_CACHE[key]


def _pack_inputs(x, squeeze_w, squeeze_b, expand1x1_w, expand1x1_b,
                 expand3x3_w, expand3x3_b, in_bf16=IN_BF16, exp_bf16=EXP_BF16):
    """Host-side packing of weights into the SBUF-ready layouts."""
    f = np.float32
    xdt = ml_dtypes.bfloat16 if in_bf16 else f
    edt = ml_dtypes.bfloat16 if exp_bf16 else f
    # wsq[p, k, m] = squeeze_w[m, 128k + p]
    wsq = np.ascontiguousarray(
        squeeze_w.T.reshape(K_TILES, 128, C_SQ)    # [k, p, m]
        .transpose(1, 0, 2)
    ).astype(xdt)
    # w1[64h + s, m] = expand1x1_w[128h + m, s]
    w1 = np.concatenate(
        [expand1x1_w[:128].T, expand1x1_w[128:].T], axis=0
    ).astype(edt)                                   # [128, 128]
    # w3[64h + s, 3dy + dx, m] = expand3x3_w[128h + m, s, dy, dx]
    w3e = expand3x3_w.reshape(2, 128, C_SQ, 9)      # [h, m, s, t]
    w3 = np.ascontiguousarray(w3e.transpose(0, 2, 3, 1)).reshape(128, 9, 128)
    w3 = w3.astype(edt)
    bsq = np.tile(squeeze_b, 2).reshape(128, 1).astype(f)
    b1 = np.ascontiguousarray(expand1x1_b.reshape(2, 128).T).astype(f)
    b3 = np.ascontiguousarray(expand3x3_b.reshape(2, 128).T).astype(f)
    # [cores, n, 128k+p, s] -> [cores, n, p, k, s] so a chunk-group load is
    # one DMA with partition-major layout
    xs = np.ascontiguousarray(
        x.reshape(N_CORES, N_IMG, K_TILES, 128, HW).transpose(0, 1, 3, 2, 4)
    ).astype(xdt)
    return xs, {"wsq": wsq, "w1": w1, "w3": w3, "bsq": bsq, "b1": b1, "b3": b3}


def _run(inputs, trace=False, in_bf16=IN_BF16, exp_bf16=EXP_BF16,
         out_bf16=OUT_BF16):
    from concourse import bass_utils

    nc = _get_nc(in_bf16, exp_bf16, out_bf16)
    xs, weights = _pack_inputs(**inputs, in_bf16=in_bf16, exp_bf16=exp_bf16)
    in_maps = [{"x": xs[c], **weights} for c in range(N_CORES)]
    res = bass_utils.run_bass_kernel_spmd(
        nc, in_maps, core_ids=list(range(N_CORES)), trace=trace
    )
    out = np.concatenate([res.results[c]["out"] for c in range(N_CORES)], axis=0)
    return out.reshape(N_TOTAL, 2 * C_E, H, W).astype(np.float32), res


def kernel(**inputs) -> np.ndarray:
    inputs = {k: np.asarray(v, dtype=np.float32) for k, v in inputs.items()}
    out, _ = _run(inputs, trace=False)
    return out


# revision 18
# speedup vs baseline: 1.4445x; 1.0404x over previous
"""Trainium2 Bass kernel for a SqueezeNet Fire module.

    x [32, 512, 56, 56] fp32
    s  = relu(squeeze_w @ x + squeeze_b)          # 1x1, 512 -> 64
    e1 = relu(expand1x1_w @ s + expand1x1_b)      # 1x1, 64 -> 256
    e3 = relu(conv3x3(s, expand3x3_w) + b)        # 3x3 pad 1, 64 -> 256
    out = concat([e1, e3], channel)               # [32, 512, 56, 56] fp32

Sharding: data-parallel over batch, 4 images per NeuronCore x 8 cores.

Per-core plan (per image, spatial flattened to 56x56=3136, chunked 7x448):
  - squeeze: 4 accumulating K=128 matmuls. The stationary weights are
    duplicated along M (64 real channels -> 128) so PSUM rows 0-63 and 64-127
    both hold S; one scalar-engine relu+bias eviction writes both halves of a
    zero-padded S buffer SS [128, 58, 58] (partitions 0-63 = copy A,
    64-127 = copy B).
  - expand1x1 / expand3x3: K=64 matmuls issued as pairs on row groups 0-63 and
    64-127 (auto tile_position from base_partition) so each pair runs
    concurrently in the PE array. expand3x3 = 9 shifted-tap matmuls
    accumulating in PSUM, taps read shifted windows of SS.
  - evictions fused bias+relu: scalar engine for squeeze + e3, vector engine
    (tensor_scalar add/max) for e1.

The kernel is HBM-traffic-bound (x in + out out), so I/O is staged in bf16
(x cast on host, output upcast on host) when the *_BF16 flags are set;
matmul operands are bf16 (squeeze) / float32r (expands, TF32-like) with fp32
PSUM accumulation.
"""

import sys

if "/opt/trn_rl_repo" not in sys.path:
    sys.path.insert(0, "/opt/trn_rl_repo")

import ml_dtypes
import numpy as np

import concourse.bass as bass
import concourse.tile as tile
from concourse import bacc, mybir

F32 = mybir.dt.float32
F32R = mybir.dt.float32r
BF16 = mybir.dt.bfloat16
RELU = mybir.ActivationFunctionType.Relu

N_CORES = 8
N_TOTAL, C_IN, H, W = 32, 512, 56, 56
N_IMG = N_TOTAL // N_CORES          # images per core
C_SQ, C_E = 64, 256                 # squeeze / expand channels
HW = H * W                          # 3136
ROWS_PER_CHUNK = 8
N_CHUNK = H // ROWS_PER_CHUNK       # 7 chunks of 8 rows
CHUNK = ROWS_PER_CHUNK * W          # 448 spatial positions per chunk
HP, WP = H + 2, W + 2               # padded S frame 58x58
K_TILES = C_IN // 128               # 4

IN_BF16 = True                      # ship x to the device as bf16
EXP_BF16 = True                     # expand path (S buffer + weights) in bf16
OUT_BF16 = False                    # write out as bf16, upcast on host


def _build(in_bf16, exp_bf16, out_bf16):
    xdt = BF16 if in_bf16 else F32R
    edt = BF16 if exp_bf16 else F32R
    odt = BF16 if out_bf16 else F32
    nc = bacc.Bacc("TRN2", target_bir_lowering=False, debug=False,
                   num_devices=N_CORES)
    x_d = nc.dram_tensor("x", [N_IMG, 128, K_TILES, HW], xdt,
                         kind="ExternalInput").ap()
    wsq_d = nc.dram_tensor("wsq", [128, K_TILES, 128], xdt,
                           kind="ExternalInput").ap()
    w1_d = nc.dram_tensor("w1", [128, 128], edt, kind="ExternalInput").ap()
    w3_d = nc.dram_tensor("w3", [128, 9, 128], edt, kind="ExternalInput").ap()
    bsq_d = nc.dram_tensor("bsq", [128, 1], F32, kind="ExternalInput").ap()
    b1_d = nc.dram_tensor("b1", [128, 2], F32, kind="ExternalInput").ap()
    b3_d = nc.dram_tensor("b3", [128, 2], F32, kind="ExternalInput").ap()
    out_d = nc.dram_tensor("out", [N_IMG, 2 * C_E, HW], odt,
                           kind="ExternalOutput").ap()

    with tile.TileContext(nc) as tc:
        with (
            tc.tile_pool(name="wpool", bufs=1) as wpool,
            tc.tile_pool(name="xpool", bufs=5) as xpool,
            tc.tile_pool(name="sspool", bufs=2) as sspool,
            tc.tile_pool(name="opool", bufs=3) as opool,
            tc.tile_pool(name="psum", bufs=1, space="PSUM") as psum,
        ):
            wsq_t = wpool.tile([128, K_TILES, 128], xdt)
            w1_t = wpool.tile([128, 128], edt)
            w3_t = wpool.tile([128, 9, 128], edt)
            bsq_t = wpool.tile([128, 1], F32)
            b1_t = wpool.tile([128, 2], F32)
            b3_t = wpool.tile([128, 2], F32)
            nc.sync.dma_start(wsq_t[:], wsq_d[:])
            nc.sync.dma_start(w1_t[:], w1_d[:])
            nc.sync.dma_start(w3_t[:], w3_d[:])
            nc.sync.dma_start(bsq_t[:], bsq_d[:])
            nc.sync.dma_start(b1_t[:], b1_d[:])
            nc.sync.dma_start(b3_t[:], b3_d[:])

            x_tiles = {}    # (image, chunk group) -> [128, K_TILES, 2*CHUNK]
            ss_tiles = {}   # image -> SS tile
            out_stage = [None] * 4

            def load_group(n, g, eng=None):
                # one DMA covers all 4 K-tiles of a chunk group (2 chunks,
                # or 1 for the image's odd last chunk); issued on the gpsimd
                # (SWDGE) queue so a slot-wait here never head-of-line-blocks
                # the sync (HWDGE) output DMAs. The first groups go on sync
                # (hardware DGE) instead: it starts transfers ~3us sooner
                # out of the preamble, and there are no output DMAs yet to
                # block.
                w = min(2 * CHUNK, HW - 2 * g * CHUNK)
                t = xpool.tile([128, K_TILES, w], xdt, tag="xc",
                               name=f"xc_{n}_{g}")
                (eng or nc.gpsimd).dma_start(
                    t[:], x_d[n, :, :, 2 * g * CHUNK : 2 * g * CHUNK + w]
                )
                x_tiles[(n, g)] = t

            def setup_image(n):
                ss = sspool.tile([128, HP, WP], edt, tag="ss")
                # zero the one-pixel border of the padded S frame (memset
                # rejects the f32r dtype tag, so write through a plain view)
                mdt = BF16 if exp_bf16 else F32
                nc.vector.memset(ss[:, 0, :].bitcast(mdt), 0.0)
                nc.vector.memset(ss[:, HP - 1, :].bitcast(mdt), 0.0)
                nc.vector.memset(ss[:, 1 : HP - 1, 0].bitcast(mdt), 0.0)
                nc.vector.memset(ss[:, 1 : HP - 1, WP - 1].bitcast(mdt), 0.0)
                ss_tiles[n] = ss

            def squeeze_chunk(n, j):
                if n not in ss_tiles:
                    setup_image(n)
                ps = psum.tile([128, ROWS_PER_CHUNK, W], F32, tag="sq", bufs=2,
                               name=f"sq_{n}_{j}")
                xt = x_tiles[(n, j // 2)]
                c0 = (j % 2) * CHUNK
                for k in range(K_TILES):
                    nc.tensor.matmul(
                        ps[:],
                        wsq_t[:, k, :],
                        xt[:, k, c0 : c0 + CHUNK],
                        start=(k == 0),
                        stop=(k == K_TILES - 1),
                    )
                # relu+bias eviction into both duplicated halves of SS interior
                y0 = j * ROWS_PER_CHUNK
                nc.scalar.activation(
                    ss_tiles[n][:, 1 + y0 : 1 + y0 + ROWS_PER_CHUNK, 1 : 1 + W],
                    ps[:],
                    RELU,
                    bias=bsq_t[:],
                )

            e_state = {}

            def expand_chunk_mm(n, j, taps):
                # taps: range of expand3x3 tap indices to emit now. Callers
                # split chunk 2g+1's taps so the dy>=1 taps (which read S
                # rows of pair g+1) are emitted late, hiding the eviction +
                # mirror-copy chain of that pair behind earlier PE work.
                ss = ss_tiles[n]
                y0 = j * ROWS_PER_CHUNK
                if taps[0] == 0:
                    p1 = [psum.tile([128, CHUNK], F32, tag=f"e1h{h}", bufs=1,
                                    name=f"p1h{h}_{n}_{j}")
                          for h in range(2)]
                    p3 = [psum.tile([128, CHUNK], F32, tag=f"e3h{h}", bufs=2,
                                    name=f"p3h{h}_{n}_{j}")
                          for h in range(2)]
                    e_state[(n, j)] = (p1, p3)
                    # expand1x1: one K=64 matmul per half, concurrent pair
                    for h in range(2):
                        nc.tensor.matmul(
                            p1[h][:],
                            w1_t[64 * h : 64 * h + 64, :],
                            ss[64 * h : 64 * h + 64,
                               1 + y0 : 1 + y0 + ROWS_PER_CHUNK, 1 : 1 + W],
                            start=True,
                            stop=True,
                        )
                p1, p3 = e_state[(n, j)]
                # expand3x3: 9 shifted taps accumulate; h0/h1 issued as pairs
                for t in taps:
                    dy, dx = t // 3, t % 3
                    for h in range(2):
                        nc.tensor.matmul(
                            p3[h][:],
                            w3_t[64 * h : 64 * h + 64, t, :],
                            ss[64 * h : 64 * h + 64,
                               y0 + dy : y0 + dy + ROWS_PER_CHUNK,
                               dx : dx + W],
                            start=(t == 0),
                            stop=(t == 8),
                        )
                if taps[-1] != 8:
                    return

            def expand_chunk_evict(n, j):
                p1, p3 = e_state.pop((n, j))
                # evictions: e1 on vector engine, e3 on scalar engine.
                # Outputs stage in 2-chunk tiles; one DMA per role per pair
                # of chunks (issued after the odd chunk's eviction).
                ss = ss_tiles
                g, half = j // 2, j % 2
                gw = 1 if j == N_CHUNK - 1 else 2   # odd last chunk: solo group
                if half == 0:
                    for role in range(4):
                        out_stage[role] = opool.tile(
                            [128, gw, CHUNK], odt, tag=f"o{role}",
                            name=f"o{role}_{n}_{g}")
                for h in range(2):
                    nc.vector.tensor_scalar(
                        out_stage[h][:, half, :], p1[h][:],
                        b1_t[:, h : h + 1], 0.0,
                        op0=mybir.AluOpType.add, op1=mybir.AluOpType.max,
                    )
                for h in range(2):
                    nc.scalar.activation(out_stage[2 + h][:, half, :],
                                         p3[h][:], RELU,
                                         bias=b3_t[:, h : h + 1])
                if half + 1 == gw:
                    for role in range(4):
                        ch0 = 128 * role
                        nc.sync.dma_start(
                            out_d[n, ch0 : ch0 + 128,
                                  2 * g * CHUNK : (2 * g + gw) * CHUNK],
                            out_stage[role][:],
                        )

            # Pipeline: squeeze runs two chunks ahead of expand — expand(i)'s
            # dy=2 taps read S rows that squeeze(i+1)'s eviction writes, so
            # squeeze(i+1) must have been evicted; running squeeze(i+2) keeps
            # the PE busy during that eviction. x is prefetched PREFETCH
            # chunks ahead at chunk granularity so the pipeline never stalls
            # on an image-sized transfer.
            PREFETCH = 3            # x prefetch depth, in 2-chunk groups
            chunks = [(n, j) for n in range(N_IMG) for j in range(N_CHUNK)]
            groups = []
            for n in range(N_IMG):
                for g in range((N_CHUNK + 1) // 2):
                    groups.append((n, g))
            for gi in range(min(PREFETCH, len(groups))):
                load_group(*groups[gi])
            next_load = PREFETCH
            ALL = list(range(9))
            squeeze_chunk(*chunks[0])
            squeeze_chunk(*chunks[1])
            for i, (n, j) in enumerate(chunks):
                if i + 2 < len(chunks):
                    n2, j2 = chunks[i + 2]
                    if (n2, j2 // 2) not in x_tiles and next_load < len(groups):
                        load_group(*groups[next_load])
                        next_load += 1
                    squeeze_chunk(n2, j2)
                expand_chunk_mm(n, j, ALL)
                expand_chunk_evict(n, j)

    nc.compile()
    return nc


_NC_CACHE = {}


def _get_nc(in_bf16=IN_BF16, exp_bf16=EXP_BF16, out_bf16=OUT_BF16):
    key = (in_bf16, exp_bf16, out_bf16)
    if key not in _NC_CACHE:
        _NC_CACHE[key] = _build(in_bf16, exp_bf16, out_bf16)
    return _NC_CACHE[key]


def _pack_inputs(x, squeeze_w, squeeze_b, expand1x1_w, expand1x1_b,
                 expand3x3_w, expand3x3_b, in_bf16=IN_BF16, exp_bf16=EXP_BF16):
    """Host-side packing of weights into the SBUF-ready layouts."""
    f = np.float32
    xdt = ml_dtypes.bfloat16 if in_bf16 else f
    edt = ml_dtypes.bfloat16 if exp_bf16 else f
    # wsq[p, k, m] = squeeze_w[m % 64, 128k + p]  (M duplicated 64 -> 128)
    wsq = np.ascontiguousarray(
        np.tile(squeeze_w, (2, 1))                 # [128, 512]
        .T.reshape(K_TILES, 128, 128)              # [k, p, m]
        .transpose(1, 0, 2)
    ).astype(xdt)
    # w1[64h + s, m] = expand1x1_w[128h + m, s]
    w1 = np.concatenate(
        [expand1x1_w[:128].T, expand1x1_w[128:].T], axis=0
    ).astype(edt)                                   # [128, 128]
    # w3[64h + s, 3dy + dx, m] = expand3x3_w[128h + m, s, dy, dx]
    w3e = expand3x3_w.reshape(2, 128, C_SQ, 9)      # [h, m, s, t]
    w3 = np.ascontiguousarray(w3e.transpose(0, 2, 3, 1)).reshape(128, 9, 128)
    w3 = w3.astype(edt)
    bsq = np.tile(squeeze_b, 2).reshape(128, 1).astype(f)
    b1 = np.ascontiguousarray(expand1x1_b.reshape(2, 128).T).astype(f)
    b3 = np.ascontiguousarray(expand3x3_b.reshape(2, 128).T).astype(f)
    # [cores, n, 128k+p, s] -> [cores, n, p, k, s] so a chunk-group load is
    # one DMA with partition-major layout
    xs = np.ascontiguousarray(
        x.reshape(N_CORES, N_IMG, K_TILES, 128, HW).transpose(0, 1, 3, 2, 4)
    ).astype(xdt)
    return xs, {"wsq": wsq, "w1": w1, "w3": w3, "bsq": bsq, "b1": b1, "b3": b3}


def _run(inputs, trace=False, in_bf16=IN_BF16, exp_bf16=EXP_BF16,
         out_bf16=OUT_BF16):
    from concourse import bass_utils

    nc = _get_nc(in_bf16, exp_bf16, out_bf16)
    xs, weights = _pack_inputs(**inputs, in_bf16=in_bf16, exp_bf16=exp_bf16)
    in_maps = [{"x": xs[c], **weights} for c in range(N_CORES)]
    res = bass_utils.run_bass_kernel_spmd(
        nc, in_maps, core_ids=list(range(N_CORES)), trace=trace
    )
    out = np.concatenate([res.results[c]["out"] for c in range(N_CORES)], axis=0)
    return out.reshape(N_TOTAL, 2 * C_E, H, W).astype(np.float32), res


def kernel(**inputs) -> np.ndarray:
    inputs = {k: np.asarray(v, dtype=np.float32) for k, v in inputs.items()}
    out, _ = _run(inputs, trace=False)
    return out


# revision 20
# speedup vs baseline: 1.4508x; 1.0044x over previous
"""Trainium2 Bass kernel for a SqueezeNet Fire module.

    x [32, 512, 56, 56] fp32
    s  = relu(squeeze_w @ x + squeeze_b)          # 1x1, 512 -> 64
    e1 = relu(expand1x1_w @ s + expand1x1_b)      # 1x1, 64 -> 256
    e3 = relu(conv3x3(s, expand3x3_w) + b)        # 3x3 pad 1, 64 -> 256
    out = concat([e1, e3], channel)               # [32, 512, 56, 56] fp32

Sharding: data-parallel over batch, 4 images per NeuronCore x 8 cores.

Per-core plan (per image, spatial flattened to 56x56=3136, chunked 7x448):
  - squeeze: 4 accumulating K=128 matmuls. The stationary weights are
    duplicated along M (64 real channels -> 128) so PSUM rows 0-63 and 64-127
    both hold S; one scalar-engine relu+bias eviction writes both halves of a
    zero-padded S buffer SS [128, 58, 58] (partitions 0-63 = copy A,
    64-127 = copy B).
  - expand1x1 / expand3x3: K=64 matmuls issued as pairs on row groups 0-63 and
    64-127 (auto tile_position from base_partition) so each pair runs
    concurrently in the PE array. expand3x3 = 9 shifted-tap matmuls
    accumulating in PSUM, taps read shifted windows of SS.
  - evictions fused bias+relu: scalar engine for squeeze + e3, vector engine
    (tensor_scalar add/max) for e1.

The kernel is HBM-traffic-bound (x in + out out), so I/O is staged in bf16
(x cast on host, output upcast on host) when the *_BF16 flags are set;
matmul operands are bf16 (squeeze) / float32r (expands, TF32-like) with fp32
PSUM accumulation.
"""

import sys

if "/opt/trn_rl_repo" not in sys.path:
    sys.path.insert(0, "/opt/trn_rl_repo")

import ml_dtypes
import numpy as np

import concourse.bass as bass
import concourse.tile as tile
from concourse import bacc, mybir

F32 = mybir.dt.float32
F32R = mybir.dt.float32r
BF16 = mybir.dt.bfloat16
RELU = mybir.ActivationFunctionType.Relu

N_CORES = 8
N_TOTAL, C_IN, H, W = 32, 512, 56, 56
N_IMG = N_TOTAL // N_CORES          # images per core
C_SQ, C_E = 64, 256                 # squeeze / expand channels
HW = H * W                          # 3136
ROWS_PER_CHUNK = 8
N_CHUNK = H // ROWS_PER_CHUNK       # 7 chunks of 8 rows
CHUNK = ROWS_PER_CHUNK * W          # 448 spatial positions per chunk
HP, WP = H + 2, W + 2               # padded S frame 58x58
K_TILES = C_IN // 128               # 4

IN_BF16 = True                      # ship x to the device as bf16
EXP_BF16 = True                     # expand path (S buffer + weights) in bf16
OUT_BF16 = True                     # write out as bf16, upcast on host


def _build(in_bf16, exp_bf16, out_bf16):
    xdt = BF16 if in_bf16 else F32R
    edt = BF16 if exp_bf16 else F32R
    odt = BF16 if out_bf16 else F32
    nc = bacc.Bacc("TRN2", target_bir_lowering=False, debug=False,
                   num_devices=N_CORES)
    x_d = nc.dram_tensor("x", [N_IMG, 128, K_TILES, HW], xdt,
                         kind="ExternalInput").ap()
    wsq_d = nc.dram_tensor("wsq", [128, K_TILES, 128], xdt,
                           kind="ExternalInput").ap()
    w1_d = nc.dram_tensor("w1", [128, 128], edt, kind="ExternalInput").ap()
    w3_d = nc.dram_tensor("w3", [128, 9, 128], edt, kind="ExternalInput").ap()
    bsq_d = nc.dram_tensor("bsq", [128, 1], F32, kind="ExternalInput").ap()
    b1_d = nc.dram_tensor("b1", [128, 2], F32, kind="ExternalInput").ap()
    b3_d = nc.dram_tensor("b3", [128, 2], F32, kind="ExternalInput").ap()
    out_d = nc.dram_tensor("out", [N_IMG, 2 * C_E, HW], odt,
                           kind="ExternalOutput").ap()

    with tile.TileContext(nc) as tc:
        with (
            tc.tile_pool(name="wpool", bufs=1) as wpool,
            tc.tile_pool(name="xpool", bufs=6) as xpool,
            tc.tile_pool(name="sspool", bufs=2) as sspool,
            tc.tile_pool(name="opool", bufs=4) as opool,
            tc.tile_pool(name="psum", bufs=1, space="PSUM") as psum,
        ):
            wsq_t = wpool.tile([128, K_TILES, 128], xdt)
            w1_t = wpool.tile([128, 128], edt)
            w3_t = wpool.tile([128, 9, 128], edt)
            bsq_t = wpool.tile([128, 1], F32)
            b1_t = wpool.tile([128, 2], F32)
            b3_t = wpool.tile([128, 2], F32)
            nc.sync.dma_start(wsq_t[:], wsq_d[:])
            nc.sync.dma_start(w1_t[:], w1_d[:])
            nc.sync.dma_start(w3_t[:], w3_d[:])
            nc.sync.dma_start(bsq_t[:], bsq_d[:])
            nc.sync.dma_start(b1_t[:], b1_d[:])
            nc.sync.dma_start(b3_t[:], b3_d[:])

            x_tiles = {}    # (image, chunk group) -> [128, K_TILES, 2*CHUNK]
            ss_tiles = {}   # image -> SS tile
            out_stage = [None] * 4

            def load_group(n, g, eng=None):
                # one DMA covers all 4 K-tiles of a chunk group (2 chunks,
                # or 1 for the image's odd last chunk); issued on the gpsimd
                # (SWDGE) queue so a slot-wait here never head-of-line-blocks
                # the sync (HWDGE) output DMAs. The first groups go on sync
                # (hardware DGE) instead: it starts transfers ~3us sooner
                # out of the preamble, and there are no output DMAs yet to
                # block.
                w = min(2 * CHUNK, HW - 2 * g * CHUNK)
                t = xpool.tile([128, K_TILES, w], xdt, tag="xc",
                               name=f"xc_{n}_{g}")
                (eng or nc.gpsimd).dma_start(
                    t[:], x_d[n, :, :, 2 * g * CHUNK : 2 * g * CHUNK + w]
                )
                x_tiles[(n, g)] = t

            def setup_image(n):
                ss = sspool.tile([128, HP, WP], edt, tag="ss")
                # zero the one-pixel border of the padded S frame (memset
                # rejects the f32r dtype tag, so write through a plain view)
                mdt = BF16 if exp_bf16 else F32
                nc.vector.memset(ss[:, 0, :].bitcast(mdt), 0.0)
                nc.vector.memset(ss[:, HP - 1, :].bitcast(mdt), 0.0)
                nc.vector.memset(ss[:, 1 : HP - 1, 0].bitcast(mdt), 0.0)
                nc.vector.memset(ss[:, 1 : HP - 1, WP - 1].bitcast(mdt), 0.0)
                ss_tiles[n] = ss

            def squeeze_chunk(n, j):
                if n not in ss_tiles:
                    setup_image(n)
                ps = psum.tile([128, ROWS_PER_CHUNK, W], F32, tag="sq", bufs=2,
                               name=f"sq_{n}_{j}")
                xt = x_tiles[(n, j // 2)]
                c0 = (j % 2) * CHUNK
                for k in range(K_TILES):
                    nc.tensor.matmul(
                        ps[:],
                        wsq_t[:, k, :],
                        xt[:, k, c0 : c0 + CHUNK],
                        start=(k == 0),
                        stop=(k == K_TILES - 1),
                    )
                # relu+bias eviction into both duplicated halves of SS interior
                y0 = j * ROWS_PER_CHUNK
                nc.scalar.activation(
                    ss_tiles[n][:, 1 + y0 : 1 + y0 + ROWS_PER_CHUNK, 1 : 1 + W],
                    ps[:],
                    RELU,
                    bias=bsq_t[:],
                )

            e_state = {}

            def expand_chunk_mm(n, j, taps):
                # taps: range of expand3x3 tap indices to emit now. Callers
                # split chunk 2g+1's taps so the dy>=1 taps (which read S
                # rows of pair g+1) are emitted late, hiding the eviction +
                # mirror-copy chain of that pair behind earlier PE work.
                ss = ss_tiles[n]
                y0 = j * ROWS_PER_CHUNK
                if taps[0] == 0:
                    p1 = [psum.tile([128, CHUNK], F32, tag=f"e1h{h}", bufs=1,
                                    name=f"p1h{h}_{n}_{j}")
                          for h in range(2)]
                    p3 = [psum.tile([128, CHUNK], F32, tag=f"e3h{h}", bufs=2,
                                    name=f"p3h{h}_{n}_{j}")
                          for h in range(2)]
                    e_state[(n, j)] = (p1, p3)
                    # expand1x1: one K=64 matmul per half, concurrent pair
                    for h in range(2):
                        nc.tensor.matmul(
                            p1[h][:],
                            w1_t[64 * h : 64 * h + 64, :],
                            ss[64 * h : 64 * h + 64,
                               1 + y0 : 1 + y0 + ROWS_PER_CHUNK, 1 : 1 + W],
                            start=True,
                            stop=True,
                        )
                p1, p3 = e_state[(n, j)]
                # expand3x3: 9 shifted taps accumulate; h0/h1 issued as pairs
                for t in taps:
                    dy, dx = t // 3, t % 3
                    for h in range(2):
                        nc.tensor.matmul(
                            p3[h][:],
                            w3_t[64 * h : 64 * h + 64, t, :],
                            ss[64 * h : 64 * h + 64,
                               y0 + dy : y0 + dy + ROWS_PER_CHUNK,
                               dx : dx + W],
                            start=(t == 0),
                            stop=(t == 8),
                        )
                if taps[-1] != 8:
                    return

            def expand_chunk_evict(n, j):
                p1, p3 = e_state.pop((n, j))
                # evictions: e1 on vector engine, e3 on scalar engine.
                # Outputs stage in 2-chunk tiles; one DMA per role per pair
                # of chunks (issued after the odd chunk's eviction).
                ss = ss_tiles
                g, half = j // 2, j % 2
                gw = 1 if j == N_CHUNK - 1 else 2   # odd last chunk: solo group
                if half == 0:
                    for role in range(4):
                        out_stage[role] = opool.tile(
                            [128, gw, CHUNK], odt, tag=f"o{role}",
                            name=f"o{role}_{n}_{g}")
                for h in range(2):
                    nc.vector.tensor_scalar(
                        out_stage[h][:, half, :], p1[h][:],
                        b1_t[:, h : h + 1], 0.0,
                        op0=mybir.AluOpType.add, op1=mybir.AluOpType.max,
                    )
                for h in range(2):
                    nc.scalar.activation(out_stage[2 + h][:, half, :],
                                         p3[h][:], RELU,
                                         bias=b3_t[:, h : h + 1])
                if half + 1 == gw:
                    for role in range(4):
                        ch0 = 128 * role
                        nc.sync.dma_start(
                            out_d[n, ch0 : ch0 + 128,
                                  2 * g * CHUNK : (2 * g + gw) * CHUNK],
                            out_stage[role][:],
                        )

            # Pipeline: squeeze runs two chunks ahead of expand — expand(i)'s
            # dy=2 taps read S rows that squeeze(i+1)'s eviction writes, so
            # squeeze(i+1) must have been evicted; running squeeze(i+2) keeps
            # the PE busy during that eviction. x is prefetched PREFETCH
            # chunks ahead at chunk granularity so the pipeline never stalls
            # on an image-sized transfer.
            PREFETCH = 4            # x prefetch depth, in 2-chunk groups
            chunks = [(n, j) for n in range(N_IMG) for j in range(N_CHUNK)]
            groups = []
            for n in range(N_IMG):
                for g in range((N_CHUNK + 1) // 2):
                    groups.append((n, g))
            for gi in range(min(PREFETCH, len(groups))):
                load_group(*groups[gi])
            next_load = PREFETCH
            ALL = list(range(9))
            squeeze_chunk(*chunks[0])
            squeeze_chunk(*chunks[1])
            for i, (n, j) in enumerate(chunks):
                if i + 2 < len(chunks):
                    n2, j2 = chunks[i + 2]
                    if (n2, j2 // 2) not in x_tiles and next_load < len(groups):
                        load_group(*groups[next_load])
                        next_load += 1
                    squeeze_chunk(n2, j2)
                expand_chunk_mm(n, j, ALL)
                expand_chunk_evict(n, j)

    nc.compile()
    return nc


_NC_CACHE = {}


def _get_nc(in_bf16=IN_BF16, exp_bf16=EXP_BF16, out_bf16=OUT_BF16):
    key = (in_bf16, exp_bf16, out_bf16)
    if key not in _NC_CACHE:
        _NC_CACHE[key] = _build(in_bf16, exp_bf16, out_bf16)
    return _NC_CACHE[key]


def _pack_inputs(x, squeeze_w, squeeze_b, expand1x1_w, expand1x1_b,
                 expand3x3_w, expand3x3_b, in_bf16=IN_BF16, exp_bf16=EXP_BF16):
    """Host-side packing of weights into the SBUF-ready layouts."""
    f = np.float32
    xdt = ml_dtypes.bfloat16 if in_bf16 else f
    edt = ml_dtypes.bfloat16 if exp_bf16 else f
    # wsq[p, k, m] = squeeze_w[m % 64, 128k + p]  (M duplicated 64 -> 128)
    wsq = np.ascontiguousarray(
        np.tile(squeeze_w, (2, 1))                 # [128, 512]
        .T.reshape(K_TILES, 128, 128)              # [k, p, m]
        .transpose(1, 0, 2)
    ).astype(xdt)
    # w1[64h + s, m] = expand1x1_w[128h + m, s]
    w1 = np.concatenate(
        [expand1x1_w[:128].T, expand1x1_w[128:].T], axis=0
    ).astype(edt)                                   # [128, 128]
    # w3[64h + s, 3dy + dx, m] = expand3x3_w[128h + m, s, dy, dx]
    w3e = expand3x3_w.reshape(2, 128, C_SQ, 9)      # [h, m, s, t]
    w3 = np.ascontiguousarray(w3e.transpose(0, 2, 3, 1)).reshape(128, 9, 128)
    w3 = w3.astype(edt)
    bsq = np.tile(squeeze_b, 2).reshape(128, 1).astype(f)
    b1 = np.ascontiguousarray(expand1x1_b.reshape(2, 128).T).astype(f)
    b3 = np.ascontiguousarray(expand3x3_b.reshape(2, 128).T).astype(f)
    # [cores, n, 128k+p, s] -> [cores, n, p, k, s] so a chunk-group load is
    # one DMA with partition-major layout
    xs = np.ascontiguousarray(
        x.reshape(N_CORES, N_IMG, K_TILES, 128, HW).transpose(0, 1, 3, 2, 4)
    ).astype(xdt)
    return xs, {"wsq": wsq, "w1": w1, "w3": w3, "bsq": bsq, "b1": b1, "b3": b3}


def _run(inputs, trace=False, in_bf16=IN_BF16, exp_bf16=EXP_BF16,
         out_bf16=OUT_BF16):
    from concourse import bass_utils

    nc = _get_nc(in_bf16, exp_bf16, out_bf16)
    xs, weights = _pack_inputs(**inputs, in_bf16=in_bf16, exp_bf16=exp_bf16)
    in_maps = [{"x": xs[c], **weights} for c in range(N_CORES)]
    res = bass_utils.run_bass_kernel_spmd(
        nc, in_maps, core_ids=list(range(N_CORES)), trace=trace
    )
    out = np.concatenate([res.results[c]["out"] for c in range(N_CORES)], axis=0)
    return out.reshape(N_TOTAL, 2 * C_E, H, W).astype(np.float32), res


def kernel(**inputs) -> np.ndarray:
    inputs = {k: np.asarray(v, dtype=np.float32) for k, v in inputs.items()}
    out, _ = _run(inputs, trace=False)
    return out
